# revision 9
# baseline (speedup 1.0000x reference)
"""Bahdanau attention kernel for Trainium2 (Bass/Tile), data-parallel over batch.

Full-shape contract: kernel(**inputs) takes the full (unsharded) numpy inputs
and returns (alpha [B, L], context [B, F]) matching the jax reference.

Math (per batch b):
  U_hidden = hidden @ U_w + U_b                      [B, H]
  W_feat   = features @ W_w + W_b                    [B, L, H]
  attn     = tanh(U_hidden[:, None, :] + W_feat)     [B, L, H]
  e        = attn @ v_w (+ v_b, dropped: softmax is shift-invariant)
  alpha    = softmax(e, axis=1)                      [B, L]
  context  = sum_l alpha[b, l] * features[b, l, :]   [B, F]

Implementation notes:
  - 8 cores, 32 batches each; batches processed in groups of GB=2 so the
    [h, m] matmul output tile is [128, 392] (fits one PSUM bank in fp32).
  - Everything is computed in the transposed orientation out^T = [h, m]:
    the (U_hidden + U_b + W_b) term is then constant per partition within a
    batch and fuses into the tanh as a per-partition ACT bias; the v-dot is
    a single M=1 matmul per h-chunk.
  - features tiles are kept SBUF-resident for the group so the context
    reduction reuses them (features is read from HBM exactly once).
  - float32r (TF32-like reduced-precision fp32 matmul) runs at 1 cycle/row
    when the moving free dim is >= 256, vs 4 cycles/row for full fp32.
  - features blocks are transposed on the PE (f must be the contraction
    (partition) dim for X @ W; fp32 DMA transpose does not exist on trn2).
"""

import numpy as np
from contextlib import ExitStack

import concourse.bass as bass
import concourse.bacc as bacc
import concourse.mybir as mybir
import concourse.tile as tile
from concourse import masks
from concourse.bass_utils import run_bass_kernel_spmd

B, L, F, H = 256, 196, 2048, 512
N_CORES = 8
B_LOC = B // N_CORES  # 32

F32 = mybir.dt.float32
F32R = mybir.dt.float32r
AF = mybir.ActivationFunctionType

FC = F // 128  # 16 f-chunks of the contraction dim
HC = H // 128  # 4 h-chunks
KC = H // 128  # 4 k-chunks for U_w's contraction
L0 = 128
L1 = L - 128   # 68
GB = 2         # batches per group
GL = GB * L    # 392 columns per group


def build_kernel(n_batches: int = B_LOC) -> bacc.Bacc:
    assert n_batches % GB == 0
    nc = bacc.Bacc(trn_type="TRN2", target_bir_lowering=False, debug=False)

    feat = nc.declare_dram_parameter("features", [n_batches, L, F], F32, isOutput=False).ap()
    hid = nc.declare_dram_parameter("hidden_states", [n_batches, H], F32, isOutput=False).ap()
    u_w = nc.declare_dram_parameter("U_w", [H, H], F32, isOutput=False).ap()
    u_b = nc.declare_dram_parameter("U_b", [H], F32, isOutput=False).ap()
    w_w = nc.declare_dram_parameter("W_w", [F, H], F32, isOutput=False).ap()
    w_b = nc.declare_dram_parameter("W_b", [H], F32, isOutput=False).ap()
    v_w = nc.declare_dram_parameter("v_w", [H], F32, isOutput=False).ap()
    alpha_out = nc.declare_dram_parameter("alpha", [n_batches, L], F32, isOutput=True).ap()
    ctx_out = nc.declare_dram_parameter("context", [n_batches, F], F32, isOutput=True).ap()

    with tile.TileContext(nc) as tc:
        with ExitStack() as ctx:
            _emit(ctx, tc, feat, hid, u_w, u_b, w_w, w_b, v_w, alpha_out, ctx_out, n_batches)
    nc.compile()
    return nc


def _emit(ctx, tc, feat, hid, u_w, u_b, w_w, w_b, v_w, alpha_out, ctx_out, n_batches):
    nc = tc.nc
    ngroups = n_batches // GB

    const = ctx.enter_context(tc.tile_pool(name="const", bufs=1))

    ident = const.tile([128, 128], F32, tag="ident")
    masks.make_identity(nc, ident[:])
    identr = const.tile([128, 128], F32R, tag="identr")
    nc.scalar.copy(identr[:], ident[:])

    # W_w as [p, fc, h]: wsb[p, fc, h] = W_w[fc*128 + p, h]
    wsb = const.tile([128, FC, H], F32R, tag="wsb")
    nc.sync.dma_start(wsb[:], w_w.rearrange("(c p) h -> p c h", p=128).bitcast(F32R))
    # U_w as [p, kc, h]
    usb = const.tile([128, KC, H], F32R, tag="usb")
    nc.sync.dma_start(usb[:], u_w.rearrange("(c p) h -> p c h", p=128).bitcast(F32R))
    # v, U_b, W_b transposed into h-chunk columns: vt[p, c] = v_w[c*128 + p]
    vt = const.tile([128, HC], F32R, tag="vt")
    nc.sync.dma_start(vt[:], v_w.rearrange("(c p) -> p c", p=128).bitcast(F32R))
    ubt = const.tile([128, HC], F32, tag="ubt")
    nc.sync.dma_start(ubt[:], u_b.rearrange("(c p) -> p c", p=128))
    wbt = const.tile([128, HC], F32, tag="wbt")
    nc.sync.dma_start(wbt[:], w_b.rearrange("(c p) -> p c", p=128))
    uwb = const.tile([128, HC], F32, tag="uwb")
    nc.vector.tensor_copy(uwb[:], ubt[:])
    nc.vector.tensor_add(uwb[:], uwb[:], wbt[:])

    hsb = const.tile([n_batches, H], F32, tag="hsb")
    nc.sync.dma_start(hsb[:], hid)

    # Single-dep warmup ops: make PE/ACT observe each producer's semaphore
    # once, so the first real consumer of (producer A, producer B) pairs does
    # not need two sync waits on one instruction (HW wait-slot limit).
    with tc.tile_pool(name="psum_warm", bufs=1, space="PSUM") as psum_warm:
        wt = psum_warm.tile([128, 16], F32, tag="wt")
        nc.tensor.transpose(wt[:2, 0:2], ident[:2, :2], ident[:2, :2])
        nc.tensor.transpose(wt[:2, 2:4].bitcast(F32R), identr[:2, :2], identr[:2, :2])
        nc.tensor.matmul(wt[:2, 4:6], wsb[:, 0, 0:2], wsb[:, 0, 0:2], start=True, stop=True)
        nc.tensor.matmul(wt[:2, 6:8], usb[:, 0, 0:2], usb[:, 0, 0:2], start=True, stop=True)
        nc.tensor.matmul(wt[:1, 8:10], vt[:, 0:1], vt[:, 0:2], start=True, stop=True)

    # bias_t[p, hc*n_batches + b] = (hidden @ U_w)[b, hc*128+p] + U_b[..] + W_b[..]
    bias_t = const.tile([128, HC * n_batches], F32, tag="bias_t")
    with tc.tile_pool(name="psum_setup", bufs=1, space="PSUM") as psum_setup:
        # hidden^T: ht[p, kc*nb + b] = hidden[b, kc*128 + p]
        ph = psum_setup.tile([128, KC * n_batches], F32, tag="ph")
        for kc in range(KC):
            nc.tensor.transpose(
                ph[:, kc * n_batches:(kc + 1) * n_batches],
                hsb[:, kc * 128:(kc + 1) * 128],
                ident[:n_batches, :n_batches],
            )
        ht = const.tile([128, KC * n_batches], F32R, tag="ht")
        nc.scalar.copy(ht[:], ph[:])

        pu = psum_setup.tile([128, HC * n_batches], F32, tag="pu")
        for hc in range(HC):
            o = hc * n_batches
            for kc in range(KC):
                nc.tensor.matmul(
                    pu[:, o:o + n_batches],
                    usb[:, kc, hc * 128:(hc + 1) * 128],
                    ht[:, kc * n_batches:(kc + 1) * n_batches],
                    start=(kc == 0), stop=(kc == KC - 1),
                )
        for hc in range(HC):
            o = hc * n_batches
            nc.vector.tensor_scalar_add(
                bias_t[:, o:o + n_batches], pu[:, o:o + n_batches], uwb[:, hc:hc + 1]
            )

    # ACT observes bias_t's DVE tick once, so the first tanh carries only
    # the PE (psum) wait.
    warm_sb = const.tile([1, 1], F32, tag="warm_sb")
    nc.scalar.copy(warm_sb[:], bias_t[:1, 0:1])

    xn_pool = ctx.enter_context(tc.tile_pool(name="xn", bufs=2 * 2 * GB))
    xt_pool = ctx.enter_context(tc.tile_pool(name="xt", bufs=3))
    attn_pool = ctx.enter_context(tc.tile_pool(name="attn", bufs=2 * HC))
    alpha_pool = ctx.enter_context(tc.tile_pool(name="alphap", bufs=2))
    small = ctx.enter_context(tc.tile_pool(name="small", bufs=2))
    ctx_pool = ctx.enter_context(tc.tile_pool(name="ctxp", bufs=3))
    psum_w_pool = ctx.enter_context(tc.tile_pool(name="psum_w", bufs=HC, space="PSUM"))
    psum_t_pool = ctx.enter_context(tc.tile_pool(name="psum_t", bufs=2, space="PSUM"))
    psum_s_pool = ctx.enter_context(tc.tile_pool(name="psum_s", bufs=2, space="PSUM"))

    for g in range(ngroups):
        b0 = GB * g

        # ---- load the group's features tiles (kept resident for context) ----
        xn = []
        for bi in range(GB):
            t0 = xn_pool.tile([128, F], F32R, tag="xn")
            nc.sync.dma_start(t0[:], feat[b0 + bi, 0:L0, :].bitcast(F32R))
            t1 = xn_pool.tile([128, F], F32R, tag="xn")
            nc.sync.dma_start(t1[:L1], feat[b0 + bi, L0:L, :].bitcast(F32R))
            xn.append((t0, t1))

        # ---- step 2: W_feat^T[h, m] accumulated over f-chunks, with PE
        #      transposes producing the [f, m] view of the features ----
        pw = [psum_w_pool.tile([128, GL], F32, tag="pw", name=f"pw{i}") for i in range(HC)]
        for fc in range(FC):
            fs = slice(fc * 128, (fc + 1) * 128)
            pt = psum_t_pool.tile([128, GL], F32R, tag="pt")
            for bi in range(GB):
                t0, t1 = xn[bi]
                col = bi * L
                nc.tensor.transpose(
                    pt[:, col:col + L0], t0[:, fs], identr[:, :],
                )
                nc.tensor.transpose(
                    pt[:, col + L0:col + L], t1[:L1, fs], identr[:L1, :L1],
                )
            xt = xt_pool.tile([128, GL], F32R, tag="xt")
            nc.scalar.copy(xt[:], pt[:])
            for hc in range(HC):
                nc.tensor.matmul(
                    pw[hc][:],
                    wsb[:, fc, hc * 128:(hc + 1) * 128],
                    xt[:],
                    start=(fc == 0), stop=(fc == FC - 1),
                )

        # ---- steps 3+: attn^T = tanh(W_feat^T + bias_t) (bias fused) ----
        attn = []
        for hc in range(HC):
            at = attn_pool.tile([128, GL], F32R, tag="attn")
            for bi in range(GB):
                cs = slice(bi * L, (bi + 1) * L)
                nc.scalar.activation(
                    at[:, cs], pw[hc][:, cs], AF.Tanh,
                    bias=bias_t[:, hc * n_batches + b0 + bi: hc * n_batches + b0 + bi + 1],
                )
            attn.append(at)

        # ---- e^T[1, m] = v^T attn^T ----
        pe = psum_s_pool.tile([1, GL], F32, tag="ps")
        for hc in range(HC):
            nc.tensor.matmul(
                pe[:], vt[:, hc:hc + 1], attn[hc][:],
                start=(hc == 0), stop=(hc == HC - 1),
            )

        # ---- softmax over l per batch (single-partition row ops) ----
        negmax = small.tile([1, GB], F32, tag="negmax")
        nc.vector.tensor_reduce(
            negmax[:], pe[:].rearrange("p (b l) -> p b l", b=GB),
            axis=mybir.AxisListType.X, op=mybir.AluOpType.max, negate=True,
        )
        alpha_sb = alpha_pool.tile([1, GL], F32, tag="alpha")
        sumexp = small.tile([1, GB], F32, tag="sumexp")
        for bi in range(GB):
            cs = slice(bi * L, (bi + 1) * L)
            nc.scalar.activation(
                alpha_sb[:, cs], pe[:, cs], AF.Exp,
                bias=negmax[:, bi:bi + 1], accum_out=sumexp[:, bi:bi + 1],
            )
        rinv = small.tile([1, GB], F32, tag="rinv")
        nc.vector.reciprocal(rinv[:], sumexp[:])
        for bi in range(GB):
            cs = slice(bi * L, (bi + 1) * L)
            nc.vector.tensor_scalar_mul(alpha_sb[:, cs], alpha_sb[:, cs], rinv[:, bi:bi + 1])
            nc.sync.dma_start(alpha_out[b0 + bi:b0 + bi + 1, :], alpha_sb[:1, cs])

        # ---- alpha^T columns for the context matmuls ----
        pa = psum_s_pool.tile([128, 2 * GB], F32, tag="ps")
        for bi in range(GB):
            nc.tensor.transpose(
                pa[:L0, 2 * bi:2 * bi + 1], alpha_sb[:1, bi * L:bi * L + L0], ident[:1, :1]
            )
            nc.tensor.transpose(
                pa[:L1, 2 * bi + 1:2 * bi + 2], alpha_sb[:1, bi * L + L0:(bi + 1) * L],
                ident[:1, :1],
            )
        alphaT = small.tile([128, 2 * GB], F32R, tag="alphaT")
        for bi in range(GB):
            nc.vector.tensor_copy(alphaT[:L0, 2 * bi:2 * bi + 1], pa[:L0, 2 * bi:2 * bi + 1])
            nc.vector.tensor_copy(alphaT[:L1, 2 * bi + 1:2 * bi + 2], pa[:L1, 2 * bi + 1:2 * bi + 2])

        # ---- step 6: context[b] = alpha[b] . features[b] (reuses xn tiles) ----
        for bi in range(GB):
            t0, t1 = xn[bi]
            csb = ctx_pool.tile([1, F], F32, tag="ctx")
            for fq in range(F // 512):
                qs = slice(fq * 512, (fq + 1) * 512)
                pc = psum_s_pool.tile([1, 512], F32, tag="ps")
                nc.tensor.matmul(
                    pc[:], alphaT[:L0, 2 * bi:2 * bi + 1],
                    t0[:, qs], start=True, stop=False,
                )
                nc.tensor.matmul(
                    pc[:], alphaT[:L1, 2 * bi + 1:2 * bi + 2],
                    t1[:L1, qs], start=False, stop=True,
                )
                nc.vector.tensor_copy(csb[:, qs], pc[:])
            nc.sync.dma_start(ctx_out[b0 + bi:b0 + bi + 1, :], csb[:1, :])


_NC_CACHE: dict = {}


def _get_nc() -> bacc.Bacc:
    if "nc" not in _NC_CACHE:
        _NC_CACHE["nc"] = build_kernel(B_LOC)
    return _NC_CACHE["nc"]


def _in_maps(inputs: dict) -> list:
    f32 = lambda a: np.ascontiguousarray(np.asarray(a, dtype=np.float32))
    feats = f32(inputs["features"])
    hs = f32(inputs["hidden_states"])
    shared = {
        "U_w": f32(inputs["U_w"]),
        "U_b": f32(inputs["U_b"]),
        "W_w": f32(inputs["W_w"]),
        "W_b": f32(inputs["W_b"]),
        "v_w": f32(inputs["v_w"]),
    }
    maps = []
    for c in range(N_CORES):
        sl = slice(c * B_LOC, (c + 1) * B_LOC)
        maps.append({"features": feats[sl], "hidden_states": hs[sl], **shared})
    return maps


def run(inputs: dict, trace: bool = False):
    """Run on 8 NeuronCores; returns (alpha, context, BassKernelResults)."""
    nc = _get_nc()
    res = run_bass_kernel_spmd(nc, _in_maps(inputs), list(range(N_CORES)), trace=trace)
    alpha = np.concatenate([r["alpha"] for r in res.results], axis=0)
    context = np.concatenate([r["context"] for r in res.results], axis=0)
    return alpha, context, res


def kernel(features, hidden_states, U_w, U_b, W_w, W_b, v_w, v_b):
    alpha, context, _ = run({
        "features": features, "hidden_states": hidden_states,
        "U_w": U_w, "U_b": U_b, "W_w": W_w, "W_b": W_b, "v_w": v_w, "v_b": v_b,
    })
    return (alpha, context)


# revision 13
# speedup vs baseline: 1.0351x; 1.0351x over previous
"""Bahdanau attention kernel for Trainium2 (Bass/Tile), data-parallel over batch.

Full-shape contract: kernel(**inputs) takes the full (unsharded) numpy inputs
and returns (alpha [B, L], context [B, F]) matching the jax reference.

Math (per batch b):
  U_hidden = hidden @ U_w + U_b                      [B, H]
  W_feat   = features @ W_w + W_b                    [B, L, H]
  attn     = tanh(U_hidden[:, None, :] + W_feat)     [B, L, H]
  e        = attn @ v_w (+ v_b, dropped: softmax is shift-invariant)
  alpha    = softmax(e, axis=1)                      [B, L]
  context  = sum_l alpha[b, l] * features[b, l, :]   [B, F]

Implementation notes:
  - 8 cores, 32 batches each; batches processed in groups of GB=2 so the
    [h, m] matmul output tile is [128, 392] (fits one PSUM bank in fp32).
  - Everything is computed in the transposed orientation out^T = [h, m]:
    the (U_hidden + U_b + W_b) term is then constant per partition within a
    batch and fuses into the tanh as a per-partition ACT bias; the v-dot is
    a single M=1 matmul per h-chunk.
  - features tiles are kept SBUF-resident for the group so the context
    reduction reuses them (features is read from HBM exactly once).
  - float32r (TF32-like reduced-precision fp32 matmul) runs at 1 cycle/row
    when the moving free dim is >= 256, vs 4 cycles/row for full fp32.
  - features blocks are transposed on the PE (f must be the contraction
    (partition) dim for X @ W; fp32 DMA transpose does not exist on trn2).
"""

import numpy as np
from contextlib import ExitStack

import concourse.bass as bass
import concourse.bacc as bacc
import concourse.mybir as mybir
import concourse.tile as tile
from concourse import masks
from concourse.bass_utils import run_bass_kernel_spmd

B, L, F, H = 256, 196, 2048, 512
N_CORES = 8
B_LOC = B // N_CORES  # 32

F32 = mybir.dt.float32
F32R = mybir.dt.float32r
AF = mybir.ActivationFunctionType

FC = F // 128  # 16 f-chunks of the contraction dim
HC = H // 128  # 4 h-chunks
KC = H // 128  # 4 k-chunks for U_w's contraction
L0 = 128
L1 = L - 128   # 68
GB = 2         # batches per group
GL = GB * L    # 392 columns per group


def build_kernel(n_batches: int = B_LOC) -> bacc.Bacc:
    assert n_batches % GB == 0
    nc = bacc.Bacc(trn_type="TRN2", target_bir_lowering=False, debug=False)

    feat = nc.declare_dram_parameter("features", [n_batches, L, F], F32, isOutput=False).ap()
    hid = nc.declare_dram_parameter("hidden_states", [n_batches, H], F32, isOutput=False).ap()
    u_w = nc.declare_dram_parameter("U_w", [H, H], F32, isOutput=False).ap()
    u_b = nc.declare_dram_parameter("U_b", [H], F32, isOutput=False).ap()
    w_w = nc.declare_dram_parameter("W_w", [F, H], F32, isOutput=False).ap()
    w_b = nc.declare_dram_parameter("W_b", [H], F32, isOutput=False).ap()
    v_w = nc.declare_dram_parameter("v_w", [H], F32, isOutput=False).ap()
    alpha_out = nc.declare_dram_parameter("alpha", [n_batches, L], F32, isOutput=True).ap()
    ctx_out = nc.declare_dram_parameter("context", [n_batches, F], F32, isOutput=True).ap()

    with tile.TileContext(nc) as tc:
        with ExitStack() as ctx:
            _emit(ctx, tc, feat, hid, u_w, u_b, w_w, w_b, v_w, alpha_out, ctx_out, n_batches)
    nc.compile()
    return nc


def _emit(ctx, tc, feat, hid, u_w, u_b, w_w, w_b, v_w, alpha_out, ctx_out, n_batches):
    nc = tc.nc
    ngroups = n_batches // GB

    const = ctx.enter_context(tc.tile_pool(name="const", bufs=1))

    ident = const.tile([128, 128], F32, tag="ident")
    masks.make_identity(nc, ident[:])
    identr = const.tile([128, 128], F32R, tag="identr")
    nc.scalar.copy(identr[:], ident[:])

    # W_w as [p, fc, h]: wsb[p, fc, h] = W_w[fc*128 + p, h]
    wsb = const.tile([128, FC, H], F32R, tag="wsb")
    nc.sync.dma_start(wsb[:], w_w.rearrange("(c p) h -> p c h", p=128).bitcast(F32R))
    # U_w as [p, kc, h]
    usb = const.tile([128, KC, H], F32R, tag="usb")
    nc.sync.dma_start(usb[:], u_w.rearrange("(c p) h -> p c h", p=128).bitcast(F32R))
    # v, U_b, W_b transposed into h-chunk columns: vt[p, c] = v_w[c*128 + p]
    vt = const.tile([128, HC], F32R, tag="vt")
    nc.sync.dma_start(vt[:], v_w.rearrange("(c p) -> p c", p=128).bitcast(F32R))
    ubt = const.tile([128, HC], F32, tag="ubt")
    nc.sync.dma_start(ubt[:], u_b.rearrange("(c p) -> p c", p=128))
    wbt = const.tile([128, HC], F32, tag="wbt")
    nc.sync.dma_start(wbt[:], w_b.rearrange("(c p) -> p c", p=128))
    uwb = const.tile([128, HC], F32, tag="uwb")
    nc.vector.tensor_copy(uwb[:], ubt[:])
    nc.vector.tensor_add(uwb[:], uwb[:], wbt[:])

    hsb = const.tile([n_batches, H], F32, tag="hsb")
    nc.sync.dma_start(hsb[:], hid)

    # Single-dep warmup ops: make PE/ACT observe each producer's semaphore
    # once, so the first real consumer of (producer A, producer B) pairs does
    # not need two sync waits on one instruction (HW wait-slot limit).
    with tc.tile_pool(name="psum_warm", bufs=1, space="PSUM") as psum_warm:
        wt = psum_warm.tile([128, 16], F32, tag="wt")
        nc.tensor.transpose(wt[:2, 0:2], ident[:2, :2], ident[:2, :2])
        nc.tensor.transpose(wt[:2, 2:4].bitcast(F32R), identr[:2, :2], identr[:2, :2])
        nc.tensor.matmul(wt[:2, 4:6], wsb[:, 0, 0:2], wsb[:, 0, 0:2], start=True, stop=True)
        nc.tensor.matmul(wt[:2, 6:8], usb[:, 0, 0:2], usb[:, 0, 0:2], start=True, stop=True)
        nc.tensor.matmul(wt[:1, 8:10], vt[:, 0:1], vt[:, 0:2], start=True, stop=True)

    # bias_t[p, hc*n_batches + b] = (hidden @ U_w)[b, hc*128+p] + U_b[..] + W_b[..]
    bias_t = const.tile([128, HC * n_batches], F32, tag="bias_t")
    with tc.tile_pool(name="psum_setup", bufs=1, space="PSUM") as psum_setup:
        # hidden^T: ht[p, kc*nb + b] = hidden[b, kc*128 + p]
        ph = psum_setup.tile([128, KC * n_batches], F32, tag="ph")
        for kc in range(KC):
            nc.tensor.transpose(
                ph[:, kc * n_batches:(kc + 1) * n_batches],
                hsb[:, kc * 128:(kc + 1) * 128],
                ident[:n_batches, :n_batches],
            )
        ht = const.tile([128, KC * n_batches], F32R, tag="ht")
        nc.scalar.copy(ht[:], ph[:])

        pu = psum_setup.tile([128, HC * n_batches], F32, tag="pu")
        for hc in range(HC):
            o = hc * n_batches
            for kc in range(KC):
                nc.tensor.matmul(
                    pu[:, o:o + n_batches],
                    usb[:, kc, hc * 128:(hc + 1) * 128],
                    ht[:, kc * n_batches:(kc + 1) * n_batches],
                    start=(kc == 0), stop=(kc == KC - 1),
                )
        for hc in range(HC):
            o = hc * n_batches
            nc.vector.tensor_scalar_add(
                bias_t[:, o:o + n_batches], pu[:, o:o + n_batches], uwb[:, hc:hc + 1]
            )

    # ACT observes bias_t's DVE tick once, so the first tanh carries only
    # the PE (psum) wait.
    warm_sb = const.tile([1, 1], F32, tag="warm_sb")
    nc.scalar.copy(warm_sb[:], bias_t[:1, 0:1])

    xn_pool = ctx.enter_context(tc.tile_pool(name="xn", bufs=3 * 2 * GB))
    xt_pool = ctx.enter_context(tc.tile_pool(name="xt", bufs=3))
    attn_pool = ctx.enter_context(tc.tile_pool(name="attn", bufs=5))
    alpha_pool = ctx.enter_context(tc.tile_pool(name="alphap", bufs=3))
    small = ctx.enter_context(tc.tile_pool(name="small", bufs=2))
    ctx_pool = ctx.enter_context(tc.tile_pool(name="ctxp", bufs=2))
    # pw (step-2 accumulators) and pc (context chunks) share one 4-bank tag:
    # pc allocations reuse the pw banks freed once the group's tanh is done.
    psum_w_pool = ctx.enter_context(tc.tile_pool(name="psum_w", bufs=HC, space="PSUM"))
    psum_t_pool = ctx.enter_context(tc.tile_pool(name="psum_t", bufs=2, space="PSUM"))
    psum_e_pool = ctx.enter_context(tc.tile_pool(name="psum_e", bufs=1, space="PSUM"))
    psum_a_pool = ctx.enter_context(tc.tile_pool(name="psum_a", bufs=1, space="PSUM"))

    # Per-group state carried from iteration g to g+1 (step 6 of group g runs
    # while group g+1's step-2 matmuls keep the PE busy, so the PE never
    # waits on the softmax chain).
    pending = None  # (b0, xn, alpha_sb)

    def do_step6(b0p, xnp, alpha_p):
        pa = psum_a_pool.tile([128, 2 * GB], F32, tag="pa")
        for bi in range(GB):
            nc.tensor.transpose(
                pa[:L0, 2 * bi:2 * bi + 1], alpha_p[:1, bi * L:bi * L + L0], ident[:1, :1]
            )
            nc.tensor.transpose(
                pa[:L1, 2 * bi + 1:2 * bi + 2], alpha_p[:1, bi * L + L0:(bi + 1) * L],
                ident[:1, :1],
            )
        alphaT = small.tile([128, 2 * GB], F32R, tag="alphaT")
        for bi in range(GB):
            nc.vector.tensor_copy(alphaT[:L0, 2 * bi:2 * bi + 1], pa[:L0, 2 * bi:2 * bi + 1])
            nc.vector.tensor_copy(alphaT[:L1, 2 * bi + 1:2 * bi + 2], pa[:L1, 2 * bi + 1:2 * bi + 2])

        for bi in range(GB):
            t0, t1 = xnp[bi]
            csb = ctx_pool.tile([1, F], F32, tag="ctx")
            for fq in range(F // 512):
                qs = slice(fq * 512, (fq + 1) * 512)
                pc = psum_w_pool.tile([1, 512], F32, tag="pw", name=f"pc_{b0p}_{bi}_{fq}")
                nc.tensor.matmul(
                    pc[:], alphaT[:L0, 2 * bi:2 * bi + 1], t0[:, qs],
                    start=True, stop=False,
                )
                nc.tensor.matmul(
                    pc[:], alphaT[:L1, 2 * bi + 1:2 * bi + 2], t1[:L1, qs],
                    start=False, stop=True,
                )
                nc.vector.tensor_copy(csb[:, qs], pc[:])
            nc.sync.dma_start(ctx_out[b0p + bi:b0p + bi + 1, :], csb[:1, :])

    for g in range(ngroups):
        b0 = GB * g

        # ---- load the group's features tiles (kept resident for context) ----
        xn = []
        for bi in range(GB):
            t0 = xn_pool.tile([128, F], F32R, tag="xn", name=f"xn0_{g}_{bi}")
            nc.sync.dma_start(t0[:], feat[b0 + bi, 0:L0, :].bitcast(F32R))
            t1 = xn_pool.tile([128, F], F32R, tag="xn", name=f"xn1_{g}_{bi}")
            nc.sync.dma_start(t1[:L1], feat[b0 + bi, L0:L, :].bitcast(F32R))
            xn.append((t0, t1))

        # ---- step 2: W_feat^T[h, m] accumulated over f-chunks, with PE
        #      transposes producing the [f, m] view of the features ----
        pw = [psum_w_pool.tile([128, GL], F32, tag="pw", name=f"pw{g}_{i}") for i in range(HC)]
        for fc in range(FC):
            fs = slice(fc * 128, (fc + 1) * 128)
            pt = psum_t_pool.tile([128, GL], F32R, tag="pt")
            for bi in range(GB):
                t0, t1 = xn[bi]
                col = bi * L
                nc.tensor.transpose(
                    pt[:, col:col + L0], t0[:, fs], identr[:, :],
                )
                nc.tensor.transpose(
                    pt[:, col + L0:col + L], t1[:L1, fs], identr[:L1, :L1],
                )
            xt = xt_pool.tile([128, GL], F32R, tag="xt")
            if fc % 2 == 0:
                nc.scalar.copy(xt[:], pt[:])
            else:
                nc.vector.tensor_copy(xt[:], pt[:])
            for hc in range(HC):
                nc.tensor.matmul(
                    pw[hc][:],
                    wsb[:, fc, hc * 128:(hc + 1) * 128],
                    xt[:],
                    start=(fc == 0), stop=(fc == FC - 1),
                )

        # ---- steps 3+: attn^T = tanh(W_feat^T + bias_t) (bias fused) ----
        attn = []
        for hc in range(HC):
            at = attn_pool.tile([128, GL], F32R, tag="attn")
            for bi in range(GB):
                cs = slice(bi * L, (bi + 1) * L)
                nc.scalar.activation(
                    at[:, cs], pw[hc][:, cs], AF.Tanh,
                    bias=bias_t[:, hc * n_batches + b0 + bi: hc * n_batches + b0 + bi + 1],
                )
            attn.append(at)

        # ---- e^T[1, m] = v^T attn^T ----
        pe = psum_e_pool.tile([1, GL], F32, tag="pe")
        for hc in range(HC):
            nc.tensor.matmul(
                pe[:], vt[:, hc:hc + 1], attn[hc][:],
                start=(hc == 0), stop=(hc == HC - 1),
            )

        # ---- step 6 for the PREVIOUS group (overlaps this group's PE work) ----
        if pending is not None:
            do_step6(*pending)

        # ---- softmax over l per batch (single-partition row ops) ----
        negmax = small.tile([1, GB], F32, tag="negmax")
        nc.vector.tensor_reduce(
            negmax[:], pe[:].rearrange("p (b l) -> p b l", b=GB),
            axis=mybir.AxisListType.X, op=mybir.AluOpType.max, negate=True,
        )
        alpha_sb = alpha_pool.tile([1, GL], F32, tag="alpha")
        sumexp = small.tile([1, GB], F32, tag="sumexp")
        for bi in range(GB):
            cs = slice(bi * L, (bi + 1) * L)
            nc.scalar.activation(
                alpha_sb[:, cs], pe[:, cs], AF.Exp,
                bias=negmax[:, bi:bi + 1], accum_out=sumexp[:, bi:bi + 1],
            )
        rinv = small.tile([1, GB], F32, tag="rinv")
        nc.vector.reciprocal(rinv[:], sumexp[:])
        for bi in range(GB):
            cs = slice(bi * L, (bi + 1) * L)
            nc.vector.tensor_scalar_mul(alpha_sb[:, cs], alpha_sb[:, cs], rinv[:, bi:bi + 1])
            nc.sync.dma_start(alpha_out[b0 + bi:b0 + bi + 1, :], alpha_sb[:1, cs])

        pending = (b0, xn, alpha_sb)

    do_step6(*pending)


_NC_CACHE: dict = {}


def _get_nc() -> bacc.Bacc:
    if "nc" not in _NC_CACHE:
        _NC_CACHE["nc"] = build_kernel(B_LOC)
    return _NC_CACHE["nc"]


def _in_maps(inputs: dict) -> list:
    f32 = lambda a: np.ascontiguousarray(np.asarray(a, dtype=np.float32))
    feats = f32(inputs["features"])
    hs = f32(inputs["hidden_states"])
    shared = {
        "U_w": f32(inputs["U_w"]),
        "U_b": f32(inputs["U_b"]),
        "W_w": f32(inputs["W_w"]),
        "W_b": f32(inputs["W_b"]),
        "v_w": f32(inputs["v_w"]),
    }
    maps = []
    for c in range(N_CORES):
        sl = slice(c * B_LOC, (c + 1) * B_LOC)
        maps.append({"features": feats[sl], "hidden_states": hs[sl], **shared})
    return maps


def run(inputs: dict, trace: bool = False):
    """Run on 8 NeuronCores; returns (alpha, context, BassKernelResults)."""
    nc = _get_nc()
    res = run_bass_kernel_spmd(nc, _in_maps(inputs), list(range(N_CORES)), trace=trace)
    alpha = np.concatenate([r["alpha"] for r in res.results], axis=0)
    context = np.concatenate([r["context"] for r in res.results], axis=0)
    return alpha, context, res


def kernel(features, hidden_states, U_w, U_b, W_w, W_b, v_w, v_b):
    alpha, context, _ = run({
        "features": features, "hidden_states": hidden_states,
        "U_w": U_w, "U_b": U_b, "W_w": W_w, "W_b": W_b, "v_w": v_w, "v_b": v_b,
    })
    return (alpha, context)


# revision 15
# speedup vs baseline: 1.3750x; 1.3284x over previous
"""Bahdanau attention kernel for Trainium2 (Bass/Tile), data-parallel over batch.

Full-shape contract: kernel(**inputs) takes the full (unsharded) numpy inputs
and returns (alpha [B, L], context [B, F]) matching the jax reference.

Math (per batch b):
  U_hidden = hidden @ U_w + U_b                      [B, H]
  W_feat   = features @ W_w + W_b                    [B, L, H]
  attn     = tanh(U_hidden[:, None, :] + W_feat)     [B, L, H]
  e        = attn @ v_w (+ v_b, dropped: softmax is shift-invariant)
  alpha    = softmax(e, axis=1)                      [B, L]
  context  = sum_l alpha[b, l] * features[b, l, :]   [B, F]

Implementation notes:
  - 8 cores, 32 batches each; batches processed in groups of GB=2 so the
    [h, m] matmul output tile is [128, 392] (fits one PSUM bank in fp32).
  - Everything is computed in the transposed orientation out^T = [h, m]:
    the (U_hidden + U_b + W_b) term is then constant per partition within a
    batch and fuses into the tanh as a per-partition ACT bias; the v-dot is
    a single M=1 matmul per h-chunk.
  - features tiles are kept SBUF-resident for the group so the context
    reduction reuses them (features is read from HBM exactly once).
  - float32r (TF32-like reduced-precision fp32 matmul) runs at 1 cycle/row
    when the moving free dim is >= 256, vs 4 cycles/row for full fp32.
  - features blocks are transposed on the PE (f must be the contraction
    (partition) dim for X @ W; fp32 DMA transpose does not exist on trn2).
"""

import numpy as np
from contextlib import ExitStack

import concourse.bass as bass
import concourse.bacc as bacc
import concourse.mybir as mybir
import concourse.tile as tile
from concourse import masks
from concourse.bass_utils import run_bass_kernel_spmd

B, L, F, H = 256, 196, 2048, 512
N_CORES = 8
B_LOC = B // N_CORES  # 32

F32 = mybir.dt.float32
F32R = mybir.dt.float32r
F16 = mybir.dt.float16
AF = mybir.ActivationFunctionType

FC = F // 128  # 16 f-chunks of the contraction dim
HC = H // 128  # 4 h-chunks
KC = H // 128  # 4 k-chunks for U_w's contraction
L0 = 128
L1 = L - 128   # 68
GB = 2         # batches per group
GL = GB * L    # 392 columns per group


def build_kernel(n_batches: int = B_LOC) -> bacc.Bacc:
    assert n_batches % GB == 0
    nc = bacc.Bacc(trn_type="TRN2", target_bir_lowering=False, debug=False)

    feat = nc.declare_dram_parameter("features", [n_batches, L, F], F32, isOutput=False).ap()
    hid = nc.declare_dram_parameter("hidden_states", [n_batches, H], F32, isOutput=False).ap()
    u_w = nc.declare_dram_parameter("U_w", [H, H], F32, isOutput=False).ap()
    u_b = nc.declare_dram_parameter("U_b", [H], F32, isOutput=False).ap()
    w_w = nc.declare_dram_parameter("W_w", [F, H], F32, isOutput=False).ap()
    w_b = nc.declare_dram_parameter("W_b", [H], F32, isOutput=False).ap()
    v_w = nc.declare_dram_parameter("v_w", [H], F32, isOutput=False).ap()
    alpha_out = nc.declare_dram_parameter("alpha", [n_batches, L], F32, isOutput=True).ap()
    ctx_out = nc.declare_dram_parameter("context", [n_batches, F], F32, isOutput=True).ap()

    with tile.TileContext(nc) as tc:
        with ExitStack() as ctx:
            _emit(ctx, tc, feat, hid, u_w, u_b, w_w, w_b, v_w, alpha_out, ctx_out, n_batches)
    nc.compile()
    return nc


def _emit(ctx, tc, feat, hid, u_w, u_b, w_w, w_b, v_w, alpha_out, ctx_out, n_batches):
    nc = tc.nc
    ngroups = n_batches // GB

    const = ctx.enter_context(tc.tile_pool(name="const", bufs=1))

    ident = const.tile([128, 128], F32, tag="ident")
    masks.make_identity(nc, ident[:])
    identh = const.tile([128, 128], F16, tag="identh")
    nc.scalar.copy(identh[:], ident[:])

    # W_w as [p, fc, h]: wsb[p, fc, h] = W_w[fc*128 + p, h]
    wsb = const.tile([128, FC, H], F16, tag="wsb")
    nc.gpsimd.dma_start(wsb[:], w_w.rearrange("(c p) h -> p c h", p=128))
    # U_w as [p, kc, h]
    usb = const.tile([128, KC, H], F32, tag="usb")
    nc.sync.dma_start(usb[:], u_w.rearrange("(c p) h -> p c h", p=128))
    # v, U_b, W_b transposed into h-chunk columns: vt[p, c] = v_w[c*128 + p]
    vt = const.tile([128, HC], F16, tag="vt")
    nc.gpsimd.dma_start(vt[:], v_w.rearrange("(c p) -> p c", p=128))
    ubt = const.tile([128, HC], F32, tag="ubt")
    nc.sync.dma_start(ubt[:], u_b.rearrange("(c p) -> p c", p=128))
    wbt = const.tile([128, HC], F32, tag="wbt")
    nc.sync.dma_start(wbt[:], w_b.rearrange("(c p) -> p c", p=128))
    uwb = const.tile([128, HC], F32, tag="uwb")
    nc.vector.tensor_copy(uwb[:], ubt[:])
    nc.vector.tensor_add(uwb[:], uwb[:], wbt[:])

    hsb = const.tile([n_batches, H], F32, tag="hsb")
    nc.sync.dma_start(hsb[:], hid)

    # bias_t[p, hc*n_batches + b] = (hidden @ U_w)[b, hc*128+p] + U_b[..] + W_b[..]
    bias_t = const.tile([128, HC * n_batches], F32, tag="bias_t")
    with tc.tile_pool(name="psum_setup", bufs=1, space="PSUM") as psum_setup:
        # hidden^T: ht[p, kc*nb + b] = hidden[b, kc*128 + p]
        ph = psum_setup.tile([128, KC * n_batches], F32, tag="ph")
        for kc in range(KC):
            nc.tensor.transpose(
                ph[:, kc * n_batches:(kc + 1) * n_batches],
                hsb[:, kc * 128:(kc + 1) * 128],
                ident[:n_batches, :n_batches],
            )
        ht = const.tile([128, KC * n_batches], F32, tag="ht")
        nc.scalar.copy(ht[:], ph[:])

        pu = psum_setup.tile([128, HC * n_batches], F32, tag="pu")
        for hc in range(HC):
            o = hc * n_batches
            for kc in range(KC):
                nc.tensor.matmul(
                    pu[:, o:o + n_batches],
                    usb[:, kc, hc * 128:(hc + 1) * 128],
                    ht[:, kc * n_batches:(kc + 1) * n_batches],
                    start=(kc == 0), stop=(kc == KC - 1),
                )
        for hc in range(HC):
            o = hc * n_batches
            nc.vector.tensor_scalar_add(
                bias_t[:, o:o + n_batches], pu[:, o:o + n_batches], uwb[:, hc:hc + 1]
            )

    xn_pool = ctx.enter_context(tc.tile_pool(name="xn", bufs=3 * 2 * GB))
    xt_pool = ctx.enter_context(tc.tile_pool(name="xt", bufs=3))
    attn_pool = ctx.enter_context(tc.tile_pool(name="attn", bufs=5))
    alpha_pool = ctx.enter_context(tc.tile_pool(name="alphap", bufs=3))
    small = ctx.enter_context(tc.tile_pool(name="small", bufs=2))
    ctx_pool = ctx.enter_context(tc.tile_pool(name="ctxp", bufs=2))
    # pw (step-2 accumulators) and pc (context chunks) share one 4-bank tag:
    # pc allocations reuse the pw banks freed once the group's tanh is done.
    psum_w_pool = ctx.enter_context(tc.tile_pool(name="psum_w", bufs=HC, space="PSUM"))
    psum_t_pool = ctx.enter_context(tc.tile_pool(name="psum_t", bufs=2, space="PSUM"))
    psum_e_pool = ctx.enter_context(tc.tile_pool(name="psum_e", bufs=1, space="PSUM"))
    psum_a_pool = ctx.enter_context(tc.tile_pool(name="psum_a", bufs=1, space="PSUM"))

    # Per-group state carried from iteration g to g+1 (step 6 of group g runs
    # while group g+1's step-2 matmuls keep the PE busy, so the PE never
    # waits on the softmax chain).
    pending = None  # (b0, xn, alpha_sb)

    def do_step6(b0p, xnp, alpha_p):
        pa = psum_a_pool.tile([128, 2 * GB], F32, tag="pa")
        for bi in range(GB):
            nc.tensor.transpose(
                pa[:L0, 2 * bi:2 * bi + 1], alpha_p[:1, bi * L:bi * L + L0], ident[:1, :1]
            )
            nc.tensor.transpose(
                pa[:L1, 2 * bi + 1:2 * bi + 2], alpha_p[:1, bi * L + L0:(bi + 1) * L],
                ident[:1, :1],
            )
        alphaT = small.tile([128, 2 * GB], F16, tag="alphaT")
        for bi in range(GB):
            nc.vector.tensor_copy(alphaT[:L0, 2 * bi:2 * bi + 1], pa[:L0, 2 * bi:2 * bi + 1])
            nc.vector.tensor_copy(alphaT[:L1, 2 * bi + 1:2 * bi + 2], pa[:L1, 2 * bi + 1:2 * bi + 2])

        for bi in range(GB):
            t0, t1 = xnp[bi]
            csb = ctx_pool.tile([1, F], F32, tag="ctx")
            for fq in range(F // 512):
                qs = slice(fq * 512, (fq + 1) * 512)
                pc = psum_w_pool.tile([1, 512], F32, tag="pw", name=f"pc_{b0p}_{bi}_{fq}")
                nc.tensor.matmul(
                    pc[:], alphaT[:L0, 2 * bi:2 * bi + 1], t0[:, qs],
                    start=True, stop=False,
                )
                nc.tensor.matmul(
                    pc[:], alphaT[:L1, 2 * bi + 1:2 * bi + 2], t1[:L1, qs],
                    start=False, stop=True,
                )
                nc.vector.tensor_copy(csb[:, qs], pc[:])
            nc.sync.dma_start(ctx_out[b0p + bi:b0p + bi + 1, :], csb[:1, :])

    for g in range(ngroups):
        b0 = GB * g

        # ---- load the group's features tiles (kept resident for context) ----
        xn = []
        for bi in range(GB):
            t0 = xn_pool.tile([128, F], F16, tag="xn", name=f"xn0_{g}_{bi}")
            nc.gpsimd.dma_start(t0[:], feat[b0 + bi, 0:L0, :])
            t1 = xn_pool.tile([128, F], F16, tag="xn", name=f"xn1_{g}_{bi}")
            nc.gpsimd.dma_start(t1[:L1], feat[b0 + bi, L0:L, :])
            xn.append((t0, t1))

        # ---- step 2: W_feat^T[h, m] accumulated over f-chunks, with PE
        #      transposes producing the [f, m] view of the features ----
        pw = [psum_w_pool.tile([128, GL], F32, tag="pw", name=f"pw{g}_{i}") for i in range(HC)]
        for fc in range(FC):
            fs = slice(fc * 128, (fc + 1) * 128)
            pt = psum_t_pool.tile([128, GL], F16, tag="pt")
            for bi in range(GB):
                t0, t1 = xn[bi]
                col = bi * L
                nc.tensor.transpose(
                    pt[:, col:col + L0], t0[:, fs], identh[:, :],
                )
                nc.tensor.transpose(
                    pt[:, col + L0:col + L], t1[:L1, fs], identh[:L1, :L1],
                )
            xt = xt_pool.tile([128, GL], F16, tag="xt")
            if fc % 2 == 0:
                nc.scalar.copy(xt[:], pt[:])
            else:
                nc.vector.tensor_copy(xt[:], pt[:])
            for hc in range(HC):
                nc.tensor.matmul(
                    pw[hc][:],
                    wsb[:, fc, hc * 128:(hc + 1) * 128],
                    xt[:],
                    start=(fc == 0), stop=(fc == FC - 1),
                )

        # ---- steps 3+: attn^T = tanh(W_feat^T + bias_t) (bias fused) ----
        attn = []
        for hc in range(HC):
            at = attn_pool.tile([128, GL], F16, tag="attn")
            for bi in range(GB):
                cs = slice(bi * L, (bi + 1) * L)
                nc.scalar.activation(
                    at[:, cs], pw[hc][:, cs], AF.Tanh,
                    bias=bias_t[:, hc * n_batches + b0 + bi: hc * n_batches + b0 + bi + 1],
                )
            attn.append(at)

        # ---- e^T[1, m] = v^T attn^T ----
        pe = psum_e_pool.tile([1, GL], F32, tag="pe")
        for hc in range(HC):
            nc.tensor.matmul(
                pe[:], vt[:, hc:hc + 1], attn[hc][:],
                start=(hc == 0), stop=(hc == HC - 1),
            )

        # ---- step 6 for the PREVIOUS group (overlaps this group's PE work) ----
        if pending is not None:
            do_step6(*pending)

        # ---- softmax over l per batch (single-partition row ops) ----
        negmax = small.tile([1, GB], F32, tag="negmax")
        nc.vector.tensor_reduce(
            negmax[:], pe[:].rearrange("p (b l) -> p b l", b=GB),
            axis=mybir.AxisListType.X, op=mybir.AluOpType.max, negate=True,
        )
        alpha_sb = alpha_pool.tile([1, GL], F32, tag="alpha")
        sumexp = small.tile([1, GB], F32, tag="sumexp")
        for bi in range(GB):
            cs = slice(bi * L, (bi + 1) * L)
            nc.scalar.activation(
                alpha_sb[:, cs], pe[:, cs], AF.Exp,
                bias=negmax[:, bi:bi + 1], accum_out=sumexp[:, bi:bi + 1],
            )
        rinv = small.tile([1, GB], F32, tag="rinv")
        nc.vector.reciprocal(rinv[:], sumexp[:])
        for bi in range(GB):
            cs = slice(bi * L, (bi + 1) * L)
            nc.vector.tensor_scalar_mul(alpha_sb[:, cs], alpha_sb[:, cs], rinv[:, bi:bi + 1])
            nc.sync.dma_start(alpha_out[b0 + bi:b0 + bi + 1, :], alpha_sb[:1, cs])

        pending = (b0, xn, alpha_sb)

    do_step6(*pending)


_NC_CACHE: dict = {}


def _get_nc() -> bacc.Bacc:
    if "nc" not in _NC_CACHE:
        _NC_CACHE["nc"] = build_kernel(B_LOC)
    return _NC_CACHE["nc"]


def _in_maps(inputs: dict) -> list:
    f32 = lambda a: np.ascontiguousarray(np.asarray(a, dtype=np.float32))
    feats = f32(inputs["features"])
    hs = f32(inputs["hidden_states"])
    shared = {
        "U_w": f32(inputs["U_w"]),
        "U_b": f32(inputs["U_b"]),
        "W_w": f32(inputs["W_w"]),
        "W_b": f32(inputs["W_b"]),
        "v_w": f32(inputs["v_w"]),
    }
    maps = []
    for c in range(N_CORES):
        sl = slice(c * B_LOC, (c + 1) * B_LOC)
        maps.append({"features": feats[sl], "hidden_states": hs[sl], **shared})
    return maps


def run(inputs: dict, trace: bool = False):
    """Run on 8 NeuronCores; returns (alpha, context, BassKernelResults)."""
    nc = _get_nc()
    res = run_bass_kernel_spmd(nc, _in_maps(inputs), list(range(N_CORES)), trace=trace)
    alpha = np.concatenate([r["alpha"] for r in res.results], axis=0)
    context = np.concatenate([r["context"] for r in res.results], axis=0)
    return alpha, context, res


def kernel(features, hidden_states, U_w, U_b, W_w, W_b, v_w, v_b):
    alpha, context, _ = run({
        "features": features, "hidden_states": hidden_states,
        "U_w": U_w, "U_b": U_b, "W_w": W_w, "W_b": W_b, "v_w": v_w, "v_b": v_b,
    })
    return (alpha, context)


# revision 16
# speedup vs baseline: 1.5431x; 1.1222x over previous
"""Bahdanau attention kernel for Trainium2 (Bass/Tile), data-parallel over batch.

Full-shape contract: kernel(**inputs) takes the full (unsharded) numpy inputs
and returns (alpha [B, L], context [B, F]) matching the jax reference.

Math (per batch b):
  U_hidden = hidden @ U_w + U_b                      [B, H]
  W_feat   = features @ W_w + W_b                    [B, L, H]
  attn     = tanh(U_hidden[:, None, :] + W_feat)     [B, L, H]
  e        = attn @ v_w (+ v_b, dropped: softmax is shift-invariant)
  alpha    = softmax(e, axis=1)                      [B, L]
  context  = sum_l alpha[b, l] * features[b, l, :]   [B, F]

Implementation notes:
  - 8 cores, 32 batches each; batches processed in groups of GB=2 so the
    [h, m] matmul output tile is [128, 392] (fits one PSUM bank in fp32).
  - Everything is computed in the transposed orientation out^T = [h, m]:
    the (U_hidden + U_b + W_b) term is then constant per partition within a
    batch and fuses into the tanh as a per-partition ACT bias; the v-dot is
    a single M=1 matmul per h-chunk.
  - features tiles are kept SBUF-resident for the group so the context
    reduction reuses them (features is read from HBM exactly once).
  - float32r (TF32-like reduced-precision fp32 matmul) runs at 1 cycle/row
    when the moving free dim is >= 256, vs 4 cycles/row for full fp32.
  - features blocks are transposed on the PE (f must be the contraction
    (partition) dim for X @ W; fp32 DMA transpose does not exist on trn2).
"""

import numpy as np
from contextlib import ExitStack

import concourse.bass as bass
import concourse.bacc as bacc
import concourse.mybir as mybir
import concourse.tile as tile
from concourse import masks
from concourse.bass_utils import run_bass_kernel_spmd

B, L, F, H = 256, 196, 2048, 512
N_CORES = 8
B_LOC = B // N_CORES  # 32

F32 = mybir.dt.float32
F32R = mybir.dt.float32r
F16 = mybir.dt.float16
AF = mybir.ActivationFunctionType

FC = F // 128  # 16 f-chunks of the contraction dim
HC = H // 128  # 4 h-chunks
KC = H // 128  # 4 k-chunks for U_w's contraction
L0 = 128
L1 = L - 128   # 68
GB = 2         # batches per group
GL = GB * L    # 392 columns per group


def build_kernel(n_batches: int = B_LOC) -> bacc.Bacc:
    assert n_batches % GB == 0
    nc = bacc.Bacc(trn_type="TRN2", target_bir_lowering=False, debug=False)

    feat = nc.declare_dram_parameter("features", [n_batches, L, F], F32, isOutput=False).ap()
    hid = nc.declare_dram_parameter("hidden_states", [n_batches, H], F32, isOutput=False).ap()
    u_w = nc.declare_dram_parameter("U_w", [H, H], F32, isOutput=False).ap()
    u_b = nc.declare_dram_parameter("U_b", [H], F32, isOutput=False).ap()
    w_w = nc.declare_dram_parameter("W_w", [F, H], F32, isOutput=False).ap()
    w_b = nc.declare_dram_parameter("W_b", [H], F32, isOutput=False).ap()
    v_w = nc.declare_dram_parameter("v_w", [H], F32, isOutput=False).ap()
    alpha_out = nc.declare_dram_parameter("alpha", [n_batches, L], F32, isOutput=True).ap()
    ctx_out = nc.declare_dram_parameter("context", [n_batches, F], F32, isOutput=True).ap()

    with tile.TileContext(nc) as tc:
        with ExitStack() as ctx:
            _emit(ctx, tc, feat, hid, u_w, u_b, w_w, w_b, v_w, alpha_out, ctx_out, n_batches)
    nc.compile()
    return nc


def _emit(ctx, tc, feat, hid, u_w, u_b, w_w, w_b, v_w, alpha_out, ctx_out, n_batches):
    nc = tc.nc
    ngroups = n_batches // GB

    const = ctx.enter_context(tc.tile_pool(name="const", bufs=1))

    ident = const.tile([128, 128], F32, tag="ident")
    masks.make_identity(nc, ident[:])
    identh = const.tile([128, 128], F16, tag="identh")
    nc.scalar.copy(identh[:], ident[:])

    # W_w as [p, fc, h]: wsb[p, fc, h] = W_w[fc*128 + p, h]
    wsb = const.tile([128, FC, H], F16, tag="wsb")
    nc.gpsimd.dma_start(wsb[:], w_w.rearrange("(c p) h -> p c h", p=128))
    # U_w as [p, kc, h]
    usb = const.tile([128, KC, H], F32, tag="usb")
    nc.sync.dma_start(usb[:], u_w.rearrange("(c p) h -> p c h", p=128))
    # v, U_b, W_b transposed into h-chunk columns: vt[p, c] = v_w[c*128 + p]
    vt = const.tile([128, HC], F16, tag="vt")
    nc.gpsimd.dma_start(vt[:], v_w.rearrange("(c p) -> p c", p=128))
    ubt = const.tile([128, HC], F32, tag="ubt")
    nc.sync.dma_start(ubt[:], u_b.rearrange("(c p) -> p c", p=128))
    wbt = const.tile([128, HC], F32, tag="wbt")
    nc.sync.dma_start(wbt[:], w_b.rearrange("(c p) -> p c", p=128))
    uwb = const.tile([128, HC], F32, tag="uwb")
    nc.vector.tensor_copy(uwb[:], ubt[:])
    nc.vector.tensor_add(uwb[:], uwb[:], wbt[:])

    hsb = const.tile([n_batches, H], F32, tag="hsb")
    nc.sync.dma_start(hsb[:], hid)

    # bias_t[p, hc*n_batches + b] = (hidden @ U_w)[b, hc*128+p] + U_b[..] + W_b[..]
    bias_t = const.tile([128, HC * n_batches], F32, tag="bias_t")
    with tc.tile_pool(name="psum_setup", bufs=1, space="PSUM") as psum_setup:
        # hidden^T: ht[p, kc*nb + b] = hidden[b, kc*128 + p]
        ph = psum_setup.tile([128, KC * n_batches], F32, tag="ph")
        for kc in range(KC):
            nc.tensor.transpose(
                ph[:, kc * n_batches:(kc + 1) * n_batches],
                hsb[:, kc * 128:(kc + 1) * 128],
                ident[:n_batches, :n_batches],
            )
        ht = const.tile([128, KC * n_batches], F32, tag="ht")
        nc.scalar.copy(ht[:], ph[:])

        pu = psum_setup.tile([128, HC * n_batches], F32, tag="pu")
        for hc in range(HC):
            o = hc * n_batches
            for kc in range(KC):
                nc.tensor.matmul(
                    pu[:, o:o + n_batches],
                    usb[:, kc, hc * 128:(hc + 1) * 128],
                    ht[:, kc * n_batches:(kc + 1) * n_batches],
                    start=(kc == 0), stop=(kc == KC - 1),
                )
        for hc in range(HC):
            o = hc * n_batches
            nc.vector.tensor_scalar_add(
                bias_t[:, o:o + n_batches], pu[:, o:o + n_batches], uwb[:, hc:hc + 1]
            )

    xn_pool = ctx.enter_context(tc.tile_pool(name="xn", bufs=3 * 2 * GB))
    xt_pool = ctx.enter_context(tc.tile_pool(name="xt", bufs=3))
    attn_pool = ctx.enter_context(tc.tile_pool(name="attn", bufs=5))
    alpha_pool = ctx.enter_context(tc.tile_pool(name="alphap", bufs=3))
    small = ctx.enter_context(tc.tile_pool(name="small", bufs=2))
    ctx_pool = ctx.enter_context(tc.tile_pool(name="ctxp", bufs=2))
    # pw (step-2 accumulators) and pc (context chunks) share one 4-bank tag:
    # pc allocations reuse the pw banks freed once the group's tanh is done.
    psum_w_pool = ctx.enter_context(tc.tile_pool(name="psum_w", bufs=HC, space="PSUM"))
    psum_t_pool = ctx.enter_context(tc.tile_pool(name="psum_t", bufs=2, space="PSUM"))
    psum_e_pool = ctx.enter_context(tc.tile_pool(name="psum_e", bufs=2, space="PSUM"))

    # Per-group state carried from iteration g to g+1 (step 6 of group g runs
    # while group g+1's step-2 matmuls keep the PE busy, so the PE never
    # waits on the softmax chain).
    pending = None  # (b0, xn, alpha_sb)

    def do_step6(b0p, xnp, alpha_p):
        pa = psum_e_pool.tile([128, 2 * GB], F32, tag="pe", name="pa")
        for bi in range(GB):
            nc.tensor.transpose(
                pa[:L0, 2 * bi:2 * bi + 1], alpha_p[:1, bi * L:bi * L + L0], ident[:1, :1]
            )
            nc.tensor.transpose(
                pa[:L1, 2 * bi + 1:2 * bi + 2], alpha_p[:1, bi * L + L0:(bi + 1) * L],
                ident[:1, :1],
            )
        alphaT = small.tile([128, 2 * GB], F16, tag="alphaT")
        for bi in range(GB):
            nc.vector.tensor_copy(alphaT[:L0, 2 * bi:2 * bi + 1], pa[:L0, 2 * bi:2 * bi + 1])
            nc.vector.tensor_copy(alphaT[:L1, 2 * bi + 1:2 * bi + 2], pa[:L1, 2 * bi + 1:2 * bi + 2])

        for bi in range(GB):
            t0, t1 = xnp[bi]
            csb = ctx_pool.tile([1, F], F32, tag="ctx")
            for fq in range(F // 512):
                qs = slice(fq * 512, (fq + 1) * 512)
                pc = psum_w_pool.tile([1, 512], F32, tag="pw", name=f"pc_{b0p}_{bi}_{fq}")
                nc.tensor.matmul(
                    pc[:], alphaT[:L0, 2 * bi:2 * bi + 1], t0[:, qs],
                    start=True, stop=False,
                )
                nc.tensor.matmul(
                    pc[:], alphaT[:L1, 2 * bi + 1:2 * bi + 2], t1[:L1, qs],
                    start=False, stop=True,
                )
                nc.vector.tensor_copy(csb[:, qs], pc[:])
            nc.sync.dma_start(ctx_out[b0p + bi:b0p + bi + 1, :], csb[:1, :])

    for g in range(ngroups):
        b0 = GB * g

        # ---- load the group's features tiles (kept resident for context) ----
        xn = []
        for bi in range(GB):
            t0 = xn_pool.tile([128, F], F16, tag="xn", name=f"xn0_{g}_{bi}")
            nc.gpsimd.dma_start(t0[:], feat[b0 + bi, 0:L0, :])
            t1 = xn_pool.tile([128, F], F16, tag="xn", name=f"xn1_{g}_{bi}")
            nc.gpsimd.dma_start(t1[:L1], feat[b0 + bi, L0:L, :])
            xn.append((t0, t1))

        # ---- step 2: W_feat^T[h, m] accumulated over f-chunks, with PE
        #      transposes producing the [f, m] view of the features ----
        pw = [psum_w_pool.tile([128, GL], F32, tag="pw", name=f"pw{g}_{i}") for i in range(HC)]

        def emit_t(fc):
            fs = slice(fc * 128, (fc + 1) * 128)
            pt = psum_t_pool.tile([128, GL], F16, tag="pt", name=f"pt{g}_{fc}")
            for bi in range(GB):
                t0, t1 = xn[bi]
                col = bi * L
                nc.tensor.transpose(pt[:, col:col + L0], t0[:, fs], identh[:, :])
                nc.tensor.transpose(pt[:, col + L0:col + L], t1[:L1, fs], identh[:L1, :L1])
            xt = xt_pool.tile([128, GL], F16, tag="xt", name=f"xt{g}_{fc}")
            if fc % 2 == 0:
                nc.scalar.copy(xt[:], pt[:])
            else:
                nc.vector.tensor_copy(xt[:], pt[:])
            return xt

        def emit_m(fc, xt):
            for hc in range(HC):
                nc.tensor.matmul(
                    pw[hc][:],
                    wsb[:, fc, hc * 128:(hc + 1) * 128],
                    xt[:],
                    start=(fc == 0), stop=(fc == FC - 1),
                )

        # one-step software pipeline: T(fc+1) issues before M(fc) so the
        # PSUM->SBUF copy latency of xt(fc) hides behind the next transposes
        xt_prev = emit_t(0)
        for fc in range(1, FC):
            xt_cur = emit_t(fc)
            emit_m(fc - 1, xt_prev)
            xt_prev = xt_cur
        emit_m(FC - 1, xt_prev)

        # ---- steps 3+: attn^T = tanh(W_feat^T + bias_t) (bias fused) ----
        attn = []
        for hc in range(HC):
            at = attn_pool.tile([128, GL], F16, tag="attn")
            for bi in range(GB):
                cs = slice(bi * L, (bi + 1) * L)
                nc.scalar.activation(
                    at[:, cs], pw[hc][:, cs], AF.Tanh,
                    bias=bias_t[:, hc * n_batches + b0 + bi: hc * n_batches + b0 + bi + 1],
                )
            attn.append(at)

        # ---- step 6 for the PREVIOUS group (overlaps this group's tanh) ----
        if pending is not None:
            do_step6(*pending)

        # ---- e^T[1, m] = v^T attn^T ----
        pe = psum_e_pool.tile([1, GL], F32, tag="pe")
        for hc in range(HC):
            nc.tensor.matmul(
                pe[:], vt[:, hc:hc + 1], attn[hc][:],
                start=(hc == 0), stop=(hc == HC - 1),
            )

        # ---- softmax over l per batch (single-partition row ops) ----
        negmax = small.tile([1, GB], F32, tag="negmax")
        nc.vector.tensor_reduce(
            negmax[:], pe[:].rearrange("p (b l) -> p b l", b=GB),
            axis=mybir.AxisListType.X, op=mybir.AluOpType.max, negate=True,
        )
        alpha_sb = alpha_pool.tile([1, GL], F32, tag="alpha")
        sumexp = small.tile([1, GB], F32, tag="sumexp")
        for bi in range(GB):
            cs = slice(bi * L, (bi + 1) * L)
            nc.scalar.activation(
                alpha_sb[:, cs], pe[:, cs], AF.Exp,
                bias=negmax[:, bi:bi + 1], accum_out=sumexp[:, bi:bi + 1],
            )
        rinv = small.tile([1, GB], F32, tag="rinv")
        nc.vector.reciprocal(rinv[:], sumexp[:])
        for bi in range(GB):
            cs = slice(bi * L, (bi + 1) * L)
            nc.vector.tensor_scalar_mul(alpha_sb[:, cs], alpha_sb[:, cs], rinv[:, bi:bi + 1])
            nc.sync.dma_start(alpha_out[b0 + bi:b0 + bi + 1, :], alpha_sb[:1, cs])

        pending = (b0, xn, alpha_sb)

    do_step6(*pending)


_NC_CACHE: dict = {}


def _get_nc() -> bacc.Bacc:
    if "nc" not in _NC_CACHE:
        _NC_CACHE["nc"] = build_kernel(B_LOC)
    return _NC_CACHE["nc"]


def _in_maps(inputs: dict) -> list:
    f32 = lambda a: np.ascontiguousarray(np.asarray(a, dtype=np.float32))
    feats = f32(inputs["features"])
    hs = f32(inputs["hidden_states"])
    shared = {
        "U_w": f32(inputs["U_w"]),
        "U_b": f32(inputs["U_b"]),
        "W_w": f32(inputs["W_w"]),
        "W_b": f32(inputs["W_b"]),
        "v_w": f32(inputs["v_w"]),
    }
    maps = []
    for c in range(N_CORES):
        sl = slice(c * B_LOC, (c + 1) * B_LOC)
        maps.append({"features": feats[sl], "hidden_states": hs[sl], **shared})
    return maps


def run(inputs: dict, trace: bool = False):
    """Run on 8 NeuronCores; returns (alpha, context, BassKernelResults)."""
    nc = _get_nc()
    res = run_bass_kernel_spmd(nc, _in_maps(inputs), list(range(N_CORES)), trace=trace)
    alpha = np.concatenate([r["alpha"] for r in res.results], axis=0)
    context = np.concatenate([r["context"] for r in res.results], axis=0)
    return alpha, context, res


def kernel(features, hidden_states, U_w, U_b, W_w, W_b, v_w, v_b):
    alpha, context, _ = run({
        "features": features, "hidden_states": hidden_states,
        "U_w": U_w, "U_b": U_b, "W_w": W_w, "W_b": W_b, "v_w": v_w, "v_b": v_b,
    })
    return (alpha, context)


# revision 17
# speedup vs baseline: 1.5480x; 1.0032x over previous
"""Bahdanau attention kernel for Trainium2 (Bass/Tile), data-parallel over batch.

Full-shape contract: kernel(**inputs) takes the full (unsharded) numpy inputs
and returns (alpha [B, L], context [B, F]) matching the jax reference.

Math (per batch b):
  U_hidden = hidden @ U_w + U_b                      [B, H]
  W_feat   = features @ W_w + W_b                    [B, L, H]
  attn     = tanh(U_hidden[:, None, :] + W_feat)     [B, L, H]
  e        = attn @ v_w (+ v_b, dropped: softmax is shift-invariant)
  alpha    = softmax(e, axis=1)                      [B, L]
  context  = sum_l alpha[b, l] * features[b, l, :]   [B, F]

Implementation notes:
  - 8 cores, 32 batches each; batches processed in groups of GB=2 so the
    [h, m] matmul output tile is [128, 392] (fits one PSUM bank in fp32).
  - Everything is computed in the transposed orientation out^T = [h, m]:
    the (U_hidden + U_b + W_b) term is then constant per partition within a
    batch and fuses into the tanh as a per-partition ACT bias; the v-dot is
    a single M=1 matmul per h-chunk.
  - features tiles are kept SBUF-resident for the group so the context
    reduction reuses them (features is read from HBM exactly once).
  - float32r (TF32-like reduced-precision fp32 matmul) runs at 1 cycle/row
    when the moving free dim is >= 256, vs 4 cycles/row for full fp32.
  - features blocks are transposed on the PE (f must be the contraction
    (partition) dim for X @ W; fp32 DMA transpose does not exist on trn2).
"""

import numpy as np
from contextlib import ExitStack

import concourse.bass as bass
import concourse.bacc as bacc
import concourse.mybir as mybir
import concourse.tile as tile
from concourse import masks
from concourse.bass_utils import run_bass_kernel_spmd

B, L, F, H = 256, 196, 2048, 512
N_CORES = 8
B_LOC = B // N_CORES  # 32

F32 = mybir.dt.float32
F32R = mybir.dt.float32r
F16 = mybir.dt.float16
AF = mybir.ActivationFunctionType

FC = F // 128  # 16 f-chunks of the contraction dim
HC = H // 128  # 4 h-chunks
KC = H // 128  # 4 k-chunks for U_w's contraction
L0 = 128
L1 = L - 128   # 68
GB = 2         # batches per group
GL = GB * L    # 392 columns per group


def build_kernel(n_batches: int = B_LOC) -> bacc.Bacc:
    assert n_batches % GB == 0
    nc = bacc.Bacc(trn_type="TRN2", target_bir_lowering=False, debug=False)

    feat = nc.declare_dram_parameter("features", [n_batches, L, F], F32, isOutput=False).ap()
    hid = nc.declare_dram_parameter("hidden_states", [n_batches, H], F32, isOutput=False).ap()
    u_w = nc.declare_dram_parameter("U_w", [H, H], F32, isOutput=False).ap()
    u_b = nc.declare_dram_parameter("U_b", [H], F32, isOutput=False).ap()
    w_w = nc.declare_dram_parameter("W_w", [F, H], F32, isOutput=False).ap()
    w_b = nc.declare_dram_parameter("W_b", [H], F32, isOutput=False).ap()
    v_w = nc.declare_dram_parameter("v_w", [H], F32, isOutput=False).ap()
    alpha_out = nc.declare_dram_parameter("alpha", [n_batches, L], F32, isOutput=True).ap()
    ctx_out = nc.declare_dram_parameter("context", [n_batches, F], F32, isOutput=True).ap()

    with tile.TileContext(nc) as tc:
        with ExitStack() as ctx:
            _emit(ctx, tc, feat, hid, u_w, u_b, w_w, w_b, v_w, alpha_out, ctx_out, n_batches)
    nc.compile()
    return nc


def _emit(ctx, tc, feat, hid, u_w, u_b, w_w, w_b, v_w, alpha_out, ctx_out, n_batches):
    nc = tc.nc
    ngroups = n_batches // GB

    const = ctx.enter_context(tc.tile_pool(name="const", bufs=1))

    ident = const.tile([128, 128], F32, tag="ident")
    masks.make_identity(nc, ident[:])
    identh = const.tile([128, 128], F16, tag="identh")
    nc.scalar.copy(identh[:], ident[:])

    # W_w as [p, fc, h]: wsb[p, fc, h] = W_w[fc*128 + p, h]  (DMAs emitted
    # after the first group's feature loads so group 0 is not starved)
    wsb = const.tile([128, FC, H], F16, tag="wsb")
    w_w_r = w_w.rearrange("(c p) h -> p c h", p=128)
    # U_w as [p, kc, h]
    usb = const.tile([128, KC, H], F32, tag="usb")
    nc.sync.dma_start(usb[:], u_w.rearrange("(c p) h -> p c h", p=128))
    # v, U_b, W_b transposed into h-chunk columns: vt[p, c] = v_w[c*128 + p]
    vt = const.tile([128, HC], F16, tag="vt")
    nc.gpsimd.dma_start(vt[:], v_w.rearrange("(c p) -> p c", p=128))
    ubt = const.tile([128, HC], F32, tag="ubt")
    nc.sync.dma_start(ubt[:], u_b.rearrange("(c p) -> p c", p=128))
    wbt = const.tile([128, HC], F32, tag="wbt")
    nc.sync.dma_start(wbt[:], w_b.rearrange("(c p) -> p c", p=128))
    uwb = const.tile([128, HC], F32, tag="uwb")
    nc.vector.tensor_copy(uwb[:], ubt[:])
    nc.vector.tensor_add(uwb[:], uwb[:], wbt[:])

    hsb = const.tile([n_batches, H], F32, tag="hsb")
    nc.sync.dma_start(hsb[:], hid)

    # bias_t[p, hc*n_batches + b] = (hidden @ U_w)[b, hc*128+p] + U_b[..] + W_b[..]
    bias_t = const.tile([128, HC * n_batches], F32, tag="bias_t")
    with tc.tile_pool(name="psum_setup", bufs=1, space="PSUM") as psum_setup:
        # hidden^T: ht[p, kc*nb + b] = hidden[b, kc*128 + p]
        ph = psum_setup.tile([128, KC * n_batches], F32, tag="ph")
        for kc in range(KC):
            nc.tensor.transpose(
                ph[:, kc * n_batches:(kc + 1) * n_batches],
                hsb[:, kc * 128:(kc + 1) * 128],
                ident[:n_batches, :n_batches],
            )
        ht = const.tile([128, KC * n_batches], F32, tag="ht")
        nc.scalar.copy(ht[:], ph[:])

        pu = psum_setup.tile([128, HC * n_batches], F32, tag="pu")
        for hc in range(HC):
            o = hc * n_batches
            for kc in range(KC):
                nc.tensor.matmul(
                    pu[:, o:o + n_batches],
                    usb[:, kc, hc * 128:(hc + 1) * 128],
                    ht[:, kc * n_batches:(kc + 1) * n_batches],
                    start=(kc == 0), stop=(kc == KC - 1),
                )
        for hc in range(HC):
            o = hc * n_batches
            nc.vector.tensor_scalar_add(
                bias_t[:, o:o + n_batches], pu[:, o:o + n_batches], uwb[:, hc:hc + 1]
            )

    xn_pool = ctx.enter_context(tc.tile_pool(name="xn", bufs=3 * 2 * GB))
    xt_pool = ctx.enter_context(tc.tile_pool(name="xt", bufs=4))
    attn_pool = ctx.enter_context(tc.tile_pool(name="attn", bufs=5))
    alpha_pool = ctx.enter_context(tc.tile_pool(name="alphap", bufs=3))
    small = ctx.enter_context(tc.tile_pool(name="small", bufs=2))
    ctx_pool = ctx.enter_context(tc.tile_pool(name="ctxp", bufs=2))
    # pw (step-2 accumulators) and pc (context chunks) share one 4-bank tag:
    # pc allocations reuse the pw banks freed once the group's tanh is done.
    psum_w_pool = ctx.enter_context(tc.tile_pool(name="psum_w", bufs=HC, space="PSUM"))
    psum_t_pool = ctx.enter_context(tc.tile_pool(name="psum_t", bufs=2, space="PSUM"))
    psum_e_pool = ctx.enter_context(tc.tile_pool(name="psum_e", bufs=2, space="PSUM"))

    def load_xn(g):
        b0 = GB * g
        tiles = []
        for bi in range(GB):
            t0 = xn_pool.tile([128, F], F16, tag="xn", name=f"xn0_{g}_{bi}")
            nc.gpsimd.dma_start(t0[:], feat[b0 + bi, 0:L0, :])
            t1 = xn_pool.tile([128, F], F16, tag="xn", name=f"xn1_{g}_{bi}")
            nc.gpsimd.dma_start(t1[:L1], feat[b0 + bi, L0:L, :])
            tiles.append((t0, t1))
        return tiles

    # group 0's features first on the SWDGE queues, then the weight chunks
    # (per-fc so matmul fc only waits for its own chunk), then group 1.
    xn_pre = {0: load_xn(0)}
    for fc in range(FC):
        nc.gpsimd.dma_start(wsb[:, fc], w_w_r[:, fc])
    if ngroups > 1:
        xn_pre[1] = load_xn(1)

    # Per-group state carried from iteration g to g+1 (step 6 of group g runs
    # while group g+1's step-2 matmuls keep the PE busy, so the PE never
    # waits on the softmax chain).
    pending = None  # (b0, xn, alpha_sb)

    def do_step6(b0p, xnp, alpha_p):
        pa = psum_e_pool.tile([128, 2 * GB], F32, tag="pe", name="pa")
        for bi in range(GB):
            nc.tensor.transpose(
                pa[:L0, 2 * bi:2 * bi + 1], alpha_p[:1, bi * L:bi * L + L0], ident[:1, :1]
            )
            nc.tensor.transpose(
                pa[:L1, 2 * bi + 1:2 * bi + 2], alpha_p[:1, bi * L + L0:(bi + 1) * L],
                ident[:1, :1],
            )
        alphaT = small.tile([128, 2 * GB], F16, tag="alphaT")
        for bi in range(GB):
            nc.vector.tensor_copy(alphaT[:L0, 2 * bi:2 * bi + 1], pa[:L0, 2 * bi:2 * bi + 1])
            nc.vector.tensor_copy(alphaT[:L1, 2 * bi + 1:2 * bi + 2], pa[:L1, 2 * bi + 1:2 * bi + 2])

        for bi in range(GB):
            t0, t1 = xnp[bi]
            csb = ctx_pool.tile([1, F], F32, tag="ctx")
            for fq in range(F // 512):
                qs = slice(fq * 512, (fq + 1) * 512)
                pc = psum_w_pool.tile([1, 512], F32, tag="pw", name=f"pc_{b0p}_{bi}_{fq}")
                nc.tensor.matmul(
                    pc[:], alphaT[:L0, 2 * bi:2 * bi + 1], t0[:, qs],
                    start=True, stop=False,
                )
                nc.tensor.matmul(
                    pc[:], alphaT[:L1, 2 * bi + 1:2 * bi + 2], t1[:L1, qs],
                    start=False, stop=True,
                )
                nc.vector.tensor_copy(csb[:, qs], pc[:])
            nc.sync.dma_start(ctx_out[b0p + bi:b0p + bi + 1, :], csb[:1, :])

    for g in range(ngroups):
        b0 = GB * g

        # ---- the group's features tiles (kept resident for context) ----
        xn = xn_pre.pop(g) if g in xn_pre else load_xn(g)

        # ---- step 2: W_feat^T[h, m] accumulated over f-chunks, with PE
        #      transposes producing the [f, m] view of the features ----
        pw = [psum_w_pool.tile([128, GL], F32, tag="pw", name=f"pw{g}_{i}") for i in range(HC)]

        def emit_t(fc):
            fs = slice(fc * 128, (fc + 1) * 128)
            pt = psum_t_pool.tile([128, GL], F16, tag="pt", name=f"pt{g}_{fc}")
            for bi in range(GB):
                t0, t1 = xn[bi]
                col = bi * L
                nc.tensor.transpose(pt[:, col:col + L0], t0[:, fs], identh[:, :])
                nc.tensor.transpose(pt[:, col + L0:col + L], t1[:L1, fs], identh[:L1, :L1])
            xt = xt_pool.tile([128, GL], F16, tag="xt", name=f"xt{g}_{fc}")
            if fc % 2 == 0:
                nc.scalar.copy(xt[:], pt[:])
            else:
                nc.vector.tensor_copy(xt[:], pt[:])
            return xt

        def emit_m(fc, xt):
            for hc in range(HC):
                nc.tensor.matmul(
                    pw[hc][:],
                    wsb[:, fc, hc * 128:(hc + 1) * 128],
                    xt[:],
                    start=(fc == 0), stop=(fc == FC - 1),
                )

        # one-step software pipeline: T(fc+1) issues before M(fc) so the
        # PSUM->SBUF copy latency of xt(fc) hides behind the next transposes
        xt_prev = emit_t(0)
        for fc in range(1, FC):
            xt_cur = emit_t(fc)
            emit_m(fc - 1, xt_prev)
            xt_prev = xt_cur
        emit_m(FC - 1, xt_prev)

        # ---- steps 3+: attn^T = tanh(W_feat^T + bias_t) (bias fused) ----
        attn = []
        for hc in range(HC):
            at = attn_pool.tile([128, GL], F16, tag="attn")
            for bi in range(GB):
                cs = slice(bi * L, (bi + 1) * L)
                nc.scalar.activation(
                    at[:, cs], pw[hc][:, cs], AF.Tanh,
                    bias=bias_t[:, hc * n_batches + b0 + bi: hc * n_batches + b0 + bi + 1],
                )
            attn.append(at)

        # ---- step 6 for the PREVIOUS group (overlaps this group's tanh) ----
        if pending is not None:
            do_step6(*pending)

        # ---- e^T[1, m] = v^T attn^T ----
        pe = psum_e_pool.tile([1, GL], F32, tag="pe")
        for hc in range(HC):
            nc.tensor.matmul(
                pe[:], vt[:, hc:hc + 1], attn[hc][:],
                start=(hc == 0), stop=(hc == HC - 1),
            )

        # ---- softmax over l per batch (single-partition row ops) ----
        negmax = small.tile([1, GB], F32, tag="negmax")
        nc.vector.tensor_reduce(
            negmax[:], pe[:].rearrange("p (b l) -> p b l", b=GB),
            axis=mybir.AxisListType.X, op=mybir.AluOpType.max, negate=True,
        )
        alpha_sb = alpha_pool.tile([1, GL], F32, tag="alpha")
        sumexp = small.tile([1, GB], F32, tag="sumexp")
        for bi in range(GB):
            cs = slice(bi * L, (bi + 1) * L)
            nc.scalar.activation(
                alpha_sb[:, cs], pe[:, cs], AF.Exp,
                bias=negmax[:, bi:bi + 1], accum_out=sumexp[:, bi:bi + 1],
            )
        rinv = small.tile([1, GB], F32, tag="rinv")
        nc.vector.reciprocal(rinv[:], sumexp[:])
        for bi in range(GB):
            cs = slice(bi * L, (bi + 1) * L)
            nc.vector.tensor_scalar_mul(alpha_sb[:, cs], alpha_sb[:, cs], rinv[:, bi:bi + 1])
            nc.sync.dma_start(alpha_out[b0 + bi:b0 + bi + 1, :], alpha_sb[:1, cs])

        pending = (b0, xn, alpha_sb)

    do_step6(*pending)


_NC_CACHE: dict = {}


def _get_nc() -> bacc.Bacc:
    if "nc" not in _NC_CACHE:
        _NC_CACHE["nc"] = build_kernel(B_LOC)
    return _NC_CACHE["nc"]


def _in_maps(inputs: dict) -> list:
    f32 = lambda a: np.ascontiguousarray(np.asarray(a, dtype=np.float32))
    feats = f32(inputs["features"])
    hs = f32(inputs["hidden_states"])
    shared = {
        "U_w": f32(inputs["U_w"]),
        "U_b": f32(inputs["U_b"]),
        "W_w": f32(inputs["W_w"]),
        "W_b": f32(inputs["W_b"]),
        "v_w": f32(inputs["v_w"]),
    }
    maps = []
    for c in range(N_CORES):
        sl = slice(c * B_LOC, (c + 1) * B_LOC)
        maps.append({"features": feats[sl], "hidden_states": hs[sl], **shared})
    return maps


def run(inputs: dict, trace: bool = False):
    """Run on 8 NeuronCores; returns (alpha, context, BassKernelResults)."""
    nc = _get_nc()
    res = run_bass_kernel_spmd(nc, _in_maps(inputs), list(range(N_CORES)), trace=trace)
    alpha = np.concatenate([r["alpha"] for r in res.results], axis=0)
    context = np.concatenate([r["context"] for r in res.results], axis=0)
    return alpha, context, res


def kernel(features, hidden_states, U_w, U_b, W_w, W_b, v_w, v_b):
    alpha, context, _ = run({
        "features": features, "hidden_states": hidden_states,
        "U_w": U_w, "U_b": U_b, "W_w": W_w, "W_b": W_b, "v_w": v_w, "v_b": v_b,
    })
    return (alpha, context)
